# revision 84
# baseline (speedup 1.0000x reference)
"""BlockNet Trainium2 kernel: data-parallel over 8 NeuronCores.

Layout strategy (per core, batch NB=256):
- Dead-pyramid elimination: the final 1x1 output only needs block3[0..2]^2,
  block2[0..6]^2, block1[0..14]^2 and x[0..46]^2 — the rest of each
  block's spatial map is never read and is not computed.
- Host pre-casts x to bf16 and transposes to xprep[(w,c), h, b]; row 192
  is a constant 1.0 row that feeds matmul-fused biases.
- Per block, j-positions are grouped (nj*cout <= 32); each (group, i)
  column set [d@0 | p@32 | g@64] is computed by k accumulating matmuls
  lhsT [K, 96] x rhs slab [K, 256] at h = s*i + kh (d = untied-minus-
  shared, p = shared conv, g = gate conv). One slab row is constant 1.0
  and the kh=0 weight plane carries all biases.
- PSUM acc tiles pack WV=4 consecutive i; the blend runs on [R, wn*256]
  shapes: sigmoid (ACT) -> d+p staged to SBUF bf16 in one copy (ACT/DVE
  2:1) -> fast mul + add (DVE) -> relu (GpSimd/DVE), writing Y tensors
  [(j,cout), i, b]. All engine accesses keep 32-aligned partition bases;
  boundary-j rows shared between windows are duplicated with small DMAs.
- All compute in bf16, PSUM accumulation in f32.
"""
import numpy as np
import ml_dtypes

import concourse.bass as bass
import concourse.mybir as mybir
import concourse.bacc as bacc
import concourse.tile as tile
from concourse.bass_utils import run_bass_kernel_spmd

N_CORES = 8
NB = 256          # batch per core
BATCH = 2048
BF16 = mybir.dt.bfloat16
F32 = mybir.dt.float32
WV = 4            # i-positions per PSUM wave

# (cin, cout, k, stride, out_hw, in_hw) per block
CFG = [(3, 4, 5, 3, 20, 64), (4, 6, 3, 2, 9, 20), (6, 16, 3, 2, 4, 9),
       (16, 32, 3, 2, 1, 4)]
# effective (needed) output range per block: only these i/j feed the output
OHE = [15, 7, 3, 1]
# j-groups per block over the needed j range, nj*cout <= 32
JGS = [[(0, 8), (8, 7)], [(0, 5), (5, 2)], [(0, 2), (2, 1)], [(0, 1)]]
# rhs window rows per (blk, jg) incl the ones/bias row (blocks 1-3)
KROW = [[79, 70], [45, 21], [31, 19], [48]]
BROW = [[78, 69], [44, 20], [30, 18], [None]]
W0S = [0, 24]     # block1 slab w-offset per jg


def _relrow(blk, jg, w, c):
    """Row of input (w, c) in the (blk, jg) rhs window."""
    if blk == 0:
        return (w - W0S[jg]) * 3 + c
    if blk == 1:
        if jg == 0:
            return w * 4 + c                  # Y1a: j 0..10
        return (w - 10) * 4 + c               # Y1b: j 10..14
    if blk == 2:
        if jg == 0:
            return w * 6 + c                  # Y2a: j2 0..4
        # Y2b: j2 5..6 at 0..11, j2=4 at 12..17
        return 12 + c if w == 4 else (w - 5) * 6 + c
    return w * 16 + c                         # Y3: j3 0..2


_CACHE = {}


def _build():
    nc = bacc.Bacc("TRN2", target_bir_lowering=False, debug=False,
                   num_devices=N_CORES)
    xprep = nc.dram_tensor("xprep", [193, 64, NB], BF16,
                           kind="ExternalInput").ap()
    wb = {}
    for blk in range(4):
        k = CFG[blk][2]
        for jg in range(len(JGS[blk])):
            wb[(blk, jg)] = nc.dram_tensor(
                f"wb{blk}_{jg}", [KROW[blk][jg], OHE[blk], k, 96], BF16,
                kind="ExternalInput").ap()
    bconst4 = nc.dram_tensor("bconst4", [32, 3], F32, kind="ExternalInput").ap()
    # row 32 of wfc carries fc_b; y4's ones row pairs with it in the FC matmul
    wfc = nc.dram_tensor("wfc", [33, 4], BF16, kind="ExternalInput").ap()
    out_d = nc.dram_tensor("out", [4, NB], F32, kind="ExternalOutput").ap()

    with tile.TileContext(nc) as tc:
        import contextlib
        ctx = contextlib.ExitStack()
        with ctx:
            pconst = ctx.enter_context(tc.tile_pool(name="const", bufs=1))
            pslab = ctx.enter_context(tc.tile_pool(name="slab", bufs=1))
            pw1 = ctx.enter_context(tc.tile_pool(name="w1", bufs=6))
            pwS = ctx.enter_context(tc.tile_pool(name="wS", bufs=1))
            pg = ctx.enter_context(tc.tile_pool(name="g", bufs=4))
            pp = ctx.enter_context(tc.tile_pool(name="p", bufs=4))
            pq = ctx.enter_context(tc.tile_pool(name="q", bufs=4))
            py = ctx.enter_context(tc.tile_pool(name="y", bufs=4))
            pps = ctx.enter_context(tc.tile_pool(name="ps", bufs=3,
                                                 space="PSUM"))
            pps4 = ctx.enter_context(tc.tile_pool(name="ps4", bufs=1,
                                                  space="PSUM"))
            ppsfc = ctx.enter_context(tc.tile_pool(name="psfc", bufs=1,
                                                   space="PSUM"))

            bconst_t = pconst.tile([32, 3], F32, tag="bconst")
            wfc_t = pconst.tile([33, 4], BF16, tag="wfc")

            # x slabs cover only the needed window: h 0..47, w 0..44
            slab0 = pslab.tile([79, 48, NB], BF16, tag="slab0")
            slab1 = pslab.tile([70, 48, NB], BF16, tag="slab1")
            slabs1 = {0: slab0, 1: slab1}

            def load_slab_chunk(jg, hc):
                st = slabs1[jg]
                r0 = (0, 72)[jg]
                nrow = (78, 69)[jg]
                nc.sync.dma_start(
                    st[0:nrow, 16 * hc:16 * (hc + 1), :],
                    xprep[r0:r0 + nrow, 16 * hc:16 * (hc + 1), :])
                if hc == 0:
                    nc.sync.dma_start(st[nrow:nrow + 1, :, :],
                                      xprep[192:193, 0:48, :])

            # Y tensors; each block-2/3 rhs window is its own tile at
            # partition 0. Y1b/Y2b boundary rows come from dup DMAs.
            Y1a = pslab.tile([45, 15, NB], BF16, tag="Y1a")
            Yjg1 = pslab.tile([28, 15, NB], BF16, tag="Yjg1")
            Y1b = pslab.tile([21, 15, NB], BF16, tag="Y1b")
            Y2a = pslab.tile([31, 7, NB], BF16, tag="Y2a")
            Y2b = pslab.tile([19, 7, NB], BF16, tag="Y2b")
            Y3 = pslab.tile([48, 3, NB], BF16, tag="Y3")
            y4 = pslab.tile([33, NB], BF16, tag="y4")

            wS = {}

            def load_ws(blk, jg):
                k = CFG[blk][2]
                K = KROW[blk][jg]
                t = pwS.tile([K, OHE[blk], k, 96], BF16, tag=f"wS{blk}_{jg}")
                nc.sync.dma_start(t[:], wb[(blk, jg)][:])
                wS[(blk, jg)] = t

            nwave = [0]

            def blend_wave(R, acc, wn, ytensor, rowbase, w0, relu_dve=False):
                g_t = pg.tile([32, WV, NB], BF16, tag="g")
                q_t = pq.tile([64, WV, NB], BF16, tag="q")
                y_t = py.tile([32, WV, NB], BF16, tag="yt")
                nwave[0] += 1
                nc.scalar.activation(g_t[0:R, 0:wn, :], acc[64:64 + R, 0:wn, :],
                                     mybir.ActivationFunctionType.Sigmoid)
                # stage d AND p to SBUF bf16 in ONE op (cost is free-size
                # only) so mul and add are all-SBUF 2-byte DVE fast-path;
                # the copy alternates ACT/DVE 2:1 to balance engine load
                p_t = pp.tile([64, WV, NB], BF16, tag="p")
                if nwave[0] % 3 == 0:
                    nc.vector.tensor_copy(p_t[0:32 + R, 0:wn, :],
                                          acc[0:32 + R, 0:wn, :])
                else:
                    nc.scalar.activation(
                        p_t[0:32 + R, 0:wn, :], acc[0:32 + R, 0:wn, :],
                        mybir.ActivationFunctionType.Identity)
                # all-SBUF TensorTensor needs EQUAL input base partitions:
                # mul reads g,d at base 0, writes q at base 32; the add then
                # reads q,p both at base 32
                nc.vector.tensor_mul(q_t[32:32 + R, 0:wn, :],
                                     g_t[0:R, 0:wn, :], p_t[0:R, 0:wn, :])
                nc.vector.tensor_add(y_t[0:R, 0:wn, :],
                                     q_t[32:32 + R, 0:wn, :],
                                     p_t[32:32 + R, 0:wn, :])
                eng = nc.vector if relu_dve else nc.gpsimd
                eng.tensor_scalar_max(
                    ytensor[rowbase:rowbase + R, w0:w0 + wn, :],
                    y_t[0:R, 0:wn, :], 0.0)

            # ---- block 1 ---- DMAs emitted just-in-time in consumption
            # order; slab chunks stream ~3 waves ahead of first use
            cin, cout, k, s, oh, iw = CFG[0]
            ohe = OHE[0]
            ydst1 = {0: (Y1a, 0), 1: (Yjg1, 0)}
            chunk_post = {(1, 0): (1, 1), (1, 4): (1, 2), (1, 8): (0, 0),
                          (1, 12): (0, 1), (0, 0): (0, 2)}
            for jg, w0 in [(g, w) for g in (1, 0) for w in range(0, 15, WV)]:
                j0, nj = JGS[0][jg]
                R = nj * cout
                wn = min(WV, ohe - w0)
                wt = pw1.tile([79, WV, 5, 96], BF16, tag="w1")
                if jg == 1 and w0 == 0:
                    # first slab chunk + weight tile split in halves so the
                    # first matmuls (i<=1 read h<=7) start ~2us earlier; the
                    # DMA queue has slack post-pyramid so nothing downstream
                    # is starved by the extra submissions
                    st = slabs1[1]
                    nc.sync.dma_start(st[0:69, 0:8, :], xprep[72:141, 0:8, :])
                    nc.sync.dma_start(st[69:70, 0:48, :],
                                      xprep[192:193, 0:48, :])
                    nc.sync.dma_start(wt[0:70, 0:2, :, :],
                                      wb[(0, 1)][:, 0:2, :, :])
                    nc.sync.dma_start(st[0:69, 8:16, :],
                                      xprep[72:141, 8:16, :])
                    nc.sync.dma_start(wt[0:70, 2:4, :, :],
                                      wb[(0, 1)][:, 2:4, :, :])
                else:
                    nc.sync.dma_start(wt[0:KROW[0][jg], 0:wn, :, :],
                                      wb[(0, jg)][:, w0:w0 + wn, :, :])
                if (jg, w0) in chunk_post:
                    load_slab_chunk(*chunk_post[(jg, w0)])
                if jg == 1 and w0 == 4:
                    # FC/block4 consts, needed only at the very end
                    nc.sync.dma_start(bconst_t[:], bconst4[:])
                    nc.sync.dma_start(wfc_t[:], wfc[:])
                acc = pps.tile([96, WV, NB], F32, tag="acc")
                for iw_ in range(wn):
                    i = w0 + iw_
                    for kh in range(k):
                        nc.tensor.matmul(acc[:, iw_, :],
                                         wt[0:KROW[0][jg], iw_, kh, :],
                                         slabs1[jg][:, s * i + kh, :],
                                         start=(kh == 0),
                                         stop=(kh == k - 1))
                yt, rb = ydst1[jg]
                blend_wave(R, acc, wn, yt, rb, w0, relu_dve=True)
                if jg == 0 and w0 == 4:
                    load_ws(1, 0)
                    nc.sync.dma_start(Y1a[44:45, :, :], xprep[192:193, 0:15, :])
                if jg == 0 and w0 == 8:
                    load_ws(1, 1)
                    nc.sync.dma_start(Y1b[20:21, :, :], xprep[192:193, 0:15, :])
                if jg == 0 and w0 == 12:
                    load_ws(2, 0)
                    load_ws(2, 1)
                    load_ws(3, 0)
                    nc.sync.dma_start(Y2a[30:31, :, :], xprep[192:193, 0:7, :])
                    nc.sync.dma_start(Y2b[18:19, :, :], xprep[192:193, 0:7, :])
                    nc.sync.dma_start(y4[32:33, :], xprep[192:193, 0, :])

            # boundary-j duplicates, h-chunked per producer wave:
            # j8..10 -> Y1a rows 32..43, j10..14 -> Y1b rows 0..19
            nc.sync.dma_start(Y1a[32:44, 0:12, :], Yjg1[0:12, 0:12, :])
            nc.sync.dma_start(Y1b[0:20, 0:12, :], Yjg1[8:28, 0:12, :])
            nc.sync.dma_start(Y1a[32:44, 12:15, :], Yjg1[0:12, 12:15, :])
            nc.sync.dma_start(Y1b[0:20, 12:15, :], Yjg1[8:28, 12:15, :])

            # ---- blocks 2-3 ----
            srcs = {(1, 0): Y1a, (1, 1): Y1b, (2, 0): Y2a, (2, 1): Y2b}
            ybase = {(1, 0): (Y2a, 0), (1, 1): (Y2b, 0),
                     (2, 0): (Y3, 0), (2, 1): (Y3, 32)}
            for blk in (1, 2):
                cin, cout, k, s, oh, iw = CFG[blk]
                ohe = OHE[blk]
                wv = WV if blk == 1 else 2
                for jg in (0, 1):
                    j0, nj = JGS[blk][jg]
                    R = nj * cout
                    src = srcs[(blk, jg)]
                    yt, rb = ybase[(blk, jg)]
                    K = KROW[blk][jg]
                    for w0 in range(0, ohe, wv):
                        wn = min(wv, ohe - w0)
                        acc = pps.tile([96, WV, NB], F32, tag="acc")
                        for iw_ in range(wn):
                            i = w0 + iw_
                            for kh in range(k):
                                nc.tensor.matmul(acc[:, iw_, :],
                                                 wS[(blk, jg)][:, i, kh, :],
                                                 src[0:K, s * i + kh, :],
                                                 start=(kh == 0),
                                                 stop=(kh == k - 1))
                        blend_wave(R, acc, wn, yt, rb, w0, relu_dve=True)
                    if blk == 1 and jg == 0:
                        # j2=4 rows for block3-jg1, chunked per producer wave
                        nc.sync.dma_start(Y2b[12:18, 0:4, :], Y2a[24:30, 0:4, :])
                        nc.sync.dma_start(Y2b[12:18, 4:7, :], Y2a[24:30, 4:7, :])

            # ---- block 4 (single output position, scalar-bias blend) ----
            acc4 = pps4.tile([96, NB], F32, tag="acc4")
            for kh in range(3):
                nc.tensor.matmul(acc4[:], wS[(3, 0)][:, 0, kh, :],
                                 Y3[0:48, kh, :],
                                 start=(kh == 0), stop=(kh == 2))
            g4 = pg.tile([32, NB], BF16, tag="g4")
            nc.scalar.activation(g4[:], acc4[64:96, :],
                                 mybir.ActivationFunctionType.Sigmoid,
                                 bias=bconst_t[:, 0:1])
            q4 = pq.tile([32, NB], BF16, tag="q4")
            y4t = py.tile([32, NB], BF16, tag="y4t")
            nc.vector.scalar_tensor_tensor(
                q4[:], acc4[0:32, :], bconst_t[:, 2:3], g4[:],
                mybir.AluOpType.add, mybir.AluOpType.mult)
            nc.vector.scalar_tensor_tensor(
                y4t[:], q4[:], bconst_t[:, 1:2], acc4[32:64, :],
                mybir.AluOpType.add, mybir.AluOpType.add)
            nc.scalar.activation(y4[0:32, :], y4t[:],
                                 mybir.ActivationFunctionType.Relu)

            # ---- FC ---- (bias rides wfc row 32 x y4's ones row)
            accfc = ppsfc.tile([4, NB], F32, tag="accfc")
            nc.tensor.matmul(accfc[:], wfc_t[:], y4[:], start=True, stop=True)
            out_t = pconst.tile([4, NB], F32, tag="outt")
            nc.vector.tensor_copy(out_t[:], accfc[:])
            nc.sync.dma_start(out_d[:], out_t[:])

    nc.compile()
    return nc


def _prep_weights(inputs):
    """Build wb{blk}_{jg} [K, OHE, k, 96] bf16 (biases in the kh=0 plane at
    the ones-row for blocks 1-3), bconst4 [32,3] f32, wfc [33,4]."""
    arrs = {}
    for blk in range(4):
        cin, cout, k, st, oh, iw = CFG[blk]
        ohe = OHE[blk]
        L = oh * oh
        ln = cin * k * k
        wu = np.asarray(inputs[f"w_uc{blk + 1}"], np.float32).reshape(L, ln, cout)
        bu = np.asarray(inputs[f"b_uc{blk + 1}"], np.float32)[0]   # [cout,oh,oh]
        wp = np.asarray(inputs[f"w_pc{blk + 1}"], np.float32)      # [cout,cin,k,k]
        bp = np.asarray(inputs[f"b_pc{blk + 1}"], np.float32)      # [cout]
        wg = np.asarray(inputs[f"w_wl{blk + 1}"], np.float32)[0]   # [cin,k,k]
        bg = np.asarray(inputs[f"b_wl{blk + 1}"], np.float32)[0]   # scalar

        for jg, (j0, nj) in enumerate(JGS[blk]):
            K = KROW[blk][jg]
            W = np.zeros((K, ohe, k, 96), np.float32)
            for jt in range(nj):
                j = j0 + jt
                c0 = jt * cout
                for kw in range(k):
                    w = st * j + kw
                    for c in range(cin):
                        row = _relrow(blk, jg, w, c)
                        kidx = c * k * k
                        for kh in range(k):
                            # untied weight row l = i*oh + j uses the FULL oh
                            un = wu[np.arange(ohe) * oh + j,
                                    kidx + kh * k + kw, :]
                            W[row, :, kh, c0:c0 + cout] = (
                                un - wp[:, c, kh, kw][None, :])
                            W[row, :, kh, 32 + c0:32 + c0 + cout] = \
                                wp[:, c, kh, kw][None, :]
                            W[row, :, kh, 64 + c0:64 + c0 + cout] = \
                                wg[c, kh, kw]
                if blk != 3:
                    br = BROW[blk][jg]
                    for i in range(ohe):
                        W[br, i, 0, c0:c0 + cout] = bu[:, i, j] - bp
                        W[br, i, 0, 32 + c0:32 + c0 + cout] = bp
                        W[br, i, 0, 64 + c0:64 + c0 + cout] = bg
            arrs[f"wb{blk}_{jg}"] = W.astype(ml_dtypes.bfloat16)

    bu4 = np.asarray(inputs["b_uc4"], np.float32)[0].reshape(32)
    bp4 = np.asarray(inputs["b_pc4"], np.float32)
    bg4 = float(np.asarray(inputs["b_wl4"], np.float32)[0])
    bconst4 = np.zeros((32, 3), np.float32)
    bconst4[:, 0] = bg4
    bconst4[:, 1] = bp4
    bconst4[:, 2] = bu4 - bp4
    arrs["bconst4"] = bconst4
    wfc33 = np.vstack([np.asarray(inputs["fc_w"], np.float32),
                       np.asarray(inputs["fc_b"], np.float32).reshape(1, 4)])
    arrs["wfc"] = wfc33.astype(ml_dtypes.bfloat16)
    return arrs


def _prep_x(x, ci):
    xc = x[ci * NB:(ci + 1) * NB]                       # [256,3,64,64]
    xprep = np.empty((193, 64, NB), ml_dtypes.bfloat16)
    xprep[0:192] = xc.transpose(3, 1, 2, 0).reshape(192, 64, NB)
    xprep[192] = 1.0
    return {"xprep": xprep}


def kernel(**inputs):
    if "nc" not in _CACHE:
        _CACHE["nc"] = _build()
    nc = _CACHE["nc"]
    warrs = _prep_weights(inputs)
    x = np.asarray(inputs["x"], np.float32)
    in_maps = []
    for ci in range(N_CORES):
        m = _prep_x(x, ci)
        m.update(warrs)
        in_maps.append(m)
    res = run_bass_kernel_spmd(nc, in_maps, core_ids=list(range(N_CORES)))
    out = np.concatenate([res.results[c]["out"].T for c in range(N_CORES)],
                         axis=0)
    return out.astype(np.float32)
